# revision 11
# baseline (speedup 1.0000x reference)
"""Multi-head attention (B=2, S=4096, D=512, H=8, HD=64, fp32) on 8 TRN2 cores.

Sharding: core c -> batch b = c//4, local head pair hp = c%4 (global heads
2*hp, 2*hp+1).  Attention is head-independent: no cross-core communication.

Design (v2 -- fp8 DoubleRow attn@V + ACT/DVE exp split):

  QT = wqT.T @ xT -> [128(2 heads x 64), S] bf16 (score scale folded into wqT)
  KT likewise.  V  = x @ Wv.T + bv, written as fp8e4m3 into a DoubleRow-
  interleaved stationary layout v_dr[128k, head, kcpair, t, m] with m in
  0..127: cols 0-63 = head dims, col 64 = 1.0 (denominator), 65-127 = 0.

  per q-group of 512 queries, per unit (kcpair j of 256 keys, head h):
    scoresT chunk: 2 bf16 matmuls -> PSUM [128, 1024] (keys t=0/t=1 halves)
    exp: either ACT activation Exp (PSUM f32 -> SBUF fp8e4m3), or a
      one-instruction DVE Schraudolph: int8 = rint(s*8*log2e + 55.2),
      bitcast to fp8e4m3.  The split fraction balances ACT vs DVE occupancy.
    attn@V: ONE DoubleRow fp8 matmul per unit: lhsT = v_dr slice
      [128, 2, 128], rhs = ex [128, 2, 512], accumulating av[h] [128, 512]
      over the 16 kcpairs.  Row 64 = softmax denominator (ones column).
      DoubleRow runs at 0.5 PE cycles/row = 4x the bf16 M=65 variant.

  DMA: per-transfer HWDGE overhead is ~625ns, so everything is batched:
  packed weight tiles (1 transfer each), xT column-blocks carrying all 4
  chunks per transfer (split across the SP and gpsimd queues), one misc
  tile for biases/ident, one output DMA per q-group.

  Startup: only K block 0 + Q block 0 + V kcpairs 0-1 precede attention;
  the other K blocks / V chunks are software-pipelined into qg 0's loop.

  Tail per qg: copy av[0:65] to SBUF (emitted early for h0), PE-transpose
  [65,128] blocks, reciprocal of row 64, scale, one [128, 512] DMA.
"""

import numpy as np

B, S, D, H = 2, 4096, 512, 8
HD = D // H          # 64
OD = 128             # output dims per core (2 heads)
QW = 512             # query group width

A_SCH = 11.541560327111707   # 8 * log2(e)
B_SCH = 55.2                 # 7*8 (fp8e4m3 bias) - 0.8 (tuned offset)

PSBIG_BUFS = 3

_CACHE = {}


def _build(s=S, rep=1, sden=32, snum=13):
    global PSBIG_BUFS
    import concourse.bacc as bacc
    import concourse.mybir as mybir
    import concourse.tile as tile

    f32 = mybir.dt.float32
    bf16 = mybir.dt.bfloat16
    fp8 = mybir.dt.float8e4
    Exp = mybir.ActivationFunctionType.Exp

    nc = bacc.Bacc(None, target_bir_lowering=False)

    xT = nc.dram_tensor("xT", [D, s], bf16, kind="ExternalInput")
    wqT = nc.dram_tensor("wqT", [128, 4 * OD], bf16, kind="ExternalInput")
    wkT = nc.dram_tensor("wkT", [128, 4 * OD], bf16, kind="ExternalInput")
    wvT = nc.dram_tensor("wvT", [128, 4 * OD], bf16, kind="ExternalInput")
    # misc: cols 0:128 bvb, 128:256 ident, 256 bk, 257 bq
    misc = nc.dram_tensor("misc", [128, 258], f32, kind="ExternalInput")
    out = nc.dram_tensor("out", [s, OD], f32, kind="ExternalOutput")

    with tile.TileContext(nc) as tc:
        with (
            tc.tile_pool(name="persist", bufs=1) as persist,
            tc.tile_pool(name="exps", bufs=6) as exps,
            tc.tile_pool(name="outsb", bufs=4) as outsb,
            tc.tile_pool(name="outt", bufs=2) as outtp,
            tc.tile_pool(name="psbig", bufs=PSBIG_BUFS, space="PSUM") as psbig,
            tc.tile_pool(name="pssmall", bufs=2, space="PSUM") as pssmall,
            tc.tile_pool(name="pstp", bufs=2, space="PSUM") as pstp,
        ):
            # Warm the ScalarE exp table while input DMAs stream.
            warm = persist.tile([1, 1], f32, name="warm", tag="warm")
            nc.vector.memset(warm[:], 0.0)
            nc.scalar.activation(warm[:], warm[:], Exp)

            wq_t = persist.tile([128, 4 * OD], bf16, name="wq_t", tag="wq")
            wk_t = persist.tile([128, 4 * OD], bf16, name="wk_t", tag="wk")
            wv_t = persist.tile([128, 4 * OD], bf16, name="wv_t", tag="wv")
            misc_t = persist.tile([128, 258], f32, name="misc_t", tag="misc")
            nc.sync.dma_start(wk_t[:], wkT[:])
            nc.sync.dma_start(misc_t[:], misc[:])
            nc.sync.dma_start(wq_t[:], wqT[:])

            # xT: one SBUF tile [128, 4*s]; each transfer carries a 512-col
            # block of all 4 chunks, split across the SP / gpsimd queues.
            xt_all = persist.tile([128, 4 * s], bf16, name="xt_all",
                                  tag="xt")
            xt_v = xt_all[:].rearrange("p (c n) -> p c n", c=4)
            xT_v = xT[:].rearrange("(c p) n -> p c n", c=4)
            nc.sync.dma_start(xt_v[:, :, 0:QW], xT_v[:, :, 0:QW])
            for b in range(1, s // QW):
                eng = nc.gpsimd if b % 2 == 1 else nc.sync
                eng.dma_start(xt_v[:, :, b * QW:(b + 1) * QW],
                              xT_v[:, :, b * QW:(b + 1) * QW])
            nc.sync.dma_start(wv_t[:], wvT[:])

            qt = persist.tile([128, s], bf16, name="qt", tag="qt")
            kt = persist.tile([128, s], bf16, name="kt", tag="kt")
            # DoubleRow-interleaved V (+ones +zeros): per head h, kcpair j,
            # t in (0,1): cols h*s + j*256 + t*128 + (0..127).
            v_dr = persist.tile([128, 2 * s], fp8, name="v_dr", tag="v_dr")

            def body():
                _emit_body(nc, tc, mybir, s, qt, kt, v_dr, xt_all,
                           wq_t, wk_t, wv_t, misc_t, out,
                           persist, exps, outsb, outtp, psbig, pssmall,
                           pstp, sden, snum)

            for _ in range(rep):
                body()

    nc.compile()
    return nc


def _emit_body(nc, tc, mybir, s, qt, kt, v_dr, xt_all, wq_t, wk_t, wv_t,
               misc_t, out, persist, exps, outsb, outtp, psbig, pssmall,
               pstp, sden, snum):
    f32 = mybir.dt.float32
    fp8 = mybir.dt.float8e4
    i8 = mybir.dt.int8
    Exp = mybir.ActivationFunctionType.Exp
    Mult = mybir.AluOpType.mult
    Add = mybir.AluOpType.add
    DR = mybir.MatmulPerfMode.DoubleRow

    qg_n = s // QW
    kc_n = s // 128         # 32 key chunks of 128
    jp_n = kc_n // 2        # 16 kcpairs of 256
    sb_n = s // QW

    bvb = misc_t[:, 0:128]
    id_t = misc_t[:, 128:256]
    bk_t = misc_t[:, 256:257]
    bq_t = misc_t[:, 257:258]

    def xts(c, lo, hi):
        return xt_all[:, c * s + lo:c * s + hi]

    def proj_qk(dst, w_t, b_t, sb, pool=None, tag="sc"):
        ps = (pool or psbig).tile([128, QW], f32, name="ps_proj", tag=tag)
        for c in range(4):
            nc.tensor.matmul(
                ps[:],
                lhsT=w_t[:, c * OD:(c + 1) * OD],
                rhs=xts(c, sb * QW, (sb + 1) * QW),
                start=(c == 0),
                stop=(c == 3),
            )
        nc.vector.tensor_scalar_add(
            dst[:, sb * QW:(sb + 1) * QW], ps[:], b_t
        )

    def proj_v(sb):
        # V projection chunk -> DoubleRow layout (via psbig so it never
        # contends with the av accumulators during attention).
        ps = psbig.tile([128, 128], f32, name="ps_vp", tag="sc")
        for c in range(4):
            nc.tensor.matmul(
                ps[:],
                lhsT=xts(c, sb * 128, (sb + 1) * 128),
                rhs=wv_t[:, c * OD:(c + 1) * OD],
                start=(c == 0),
                stop=(c == 3),
            )
        # one strided add writes both heads' 64 dims (fp8 out)
        col = (sb // 2) * 256 + (sb % 2) * 128
        nc.vector.tensor_tensor(
            v_dr[:].rearrange("p (h c) -> p h c", h=2)[:, :, col:col + 64],
            ps[:].rearrange("p (h c) -> p h c", h=2),
            bvb.rearrange("p (h c) -> p h c", h=2),
            Add,
        )

    # Minimal prelude: K block 0, Q block 0, V chunks 0-3 (kcpairs 0-1).
    # Remaining K blocks / V chunks (and Q block 1) are software-pipelined
    # into qg 0's unit loop just ahead of first use, so ScalarE starts
    # exp'ing early.
    nc.gpsimd.memset(v_dr[:], 0.0)
    nc.gpsimd.memset(
        v_dr[:].rearrange("p (n m) -> p n m", m=128)[:, :, 64:65], 1.0
    )
    proj_qk(kt, wk_t, bk_t, 0)
    proj_qk(qt, wq_t, bq_t, 0)
    for sb in (0, 1, 2, 3):
        proj_v(sb)

    def emit_av(av, ex, u):
        j, h = u >> 1, u & 1
        nc.tensor.matmul(
            av[h][:],
            lhsT=v_dr[:].rearrange(
                "p (h2 j2 t m) -> p h2 j2 t m", h2=2, j2=jp_n, t=2
            )[:, h, j],
            rhs=ex[:].rearrange("p (t n) -> p t n", t=2),
            start=(j == 0),
            stop=(j == jp_n - 1),
            perf_mode=DR,
        )

    def emit_scores_exp(qg, u):
        j, h = u >> 1, u & 1
        ps = psbig.tile([128, 2 * QW], f32, name="ps_sc", tag="sc")
        for t in (0, 1):
            kc = 2 * j + t
            nc.tensor.matmul(
                ps[:, t * QW:(t + 1) * QW],
                lhsT=kt[h * HD:(h + 1) * HD,
                        kc * 128:(kc + 1) * 128],
                rhs=qt[h * HD:(h + 1) * HD,
                       qg * QW:(qg + 1) * QW],
                start=True,
                stop=True,
            )
        ex = exps.tile([128, 2 * QW], fp8, name="ex", tag="exp")
        if qg == 0:
            dve_exp = (u % 16) == 7
        else:
            dve_exp = (u * snum) % sden < snum
        if dve_exp:
            nc.vector.tensor_scalar(
                ex[:].bitcast(i8), ps[:], A_SCH, B_SCH, Mult, Add
            )
        else:
            nc.scalar.activation(ex[:], ps[:], Exp)
        return ex

    # ---- attention ----
    # Software-pipelined: each unit's attn@V matmul is emitted one unit
    # late so the PE computes the next scores while the exp runs; unit 0's
    # scores+exp are peeled into the previous q-group's tail.
    peeled_ex = None
    for qg in range(qg_n):
        av = [pssmall.tile([128, QW], f32, name="av", tag="av")
              for _ in (0, 1)]
        outt_h = [None, None]

        def tail_copy(h):
            outt = outtp.tile([65, QW], f32, name="outt", tag="outt")
            nc.vector.tensor_copy(outt[:], av[h][0:65, :])
            outt_h[h] = outt

        def tail_norm(h, ot):
            outt = outt_h[h]
            for blk in range(4):
                tp = pssmall.tile([128, 65], f32, name="tp", tag="av")
                nc.tensor.transpose(
                    tp[:],
                    outt[:, blk * 128:(blk + 1) * 128],
                    id_t[0:65, 0:65],
                )
                rs = outsb.tile([128, 1], f32, name="rs", tag="rs")
                nc.vector.reciprocal(rs[:], tp[:, 64:65])
                nc.vector.tensor_scalar_mul(
                    ot[:, blk * 128 + h * HD:blk * 128 + (h + 1) * HD],
                    tp[:, 0:64], rs[:]
                )

        pend = None
        for u in range(2 * jp_n):
            if qg == 0:
                if u % 4 == 0 and 1 <= u // 4 + 1 <= sb_n - 1:
                    proj_qk(kt, wk_t, bk_t, u // 4 + 1)
                if u == 2:
                    proj_qk(qt, wq_t, bq_t, 1)
                if u % 2 == 0 and u + 4 < kc_n:
                    proj_v(u + 4)
                    proj_v(u + 5)
            if u == 0 and peeled_ex is not None:
                ex = peeled_ex
                peeled_ex = None
            else:
                ex = emit_scores_exp(qg, u)
            if pend is not None:
                emit_av(av, *pend)
                if pend[1] == 2 * jp_n - 2:
                    # h0 accumulation done: its PSUM->SBUF copy overlaps
                    # the last h1 unit.
                    tail_copy(0)
            pend = (ex, u)
        emit_av(av, *pend)
        tail_copy(1)
        # Peel next q-group's first scores+exp so ScalarE keeps running
        # through the tail (its qt block was projected two tails ago).
        if qg + 1 < qg_n:
            peeled_ex = emit_scores_exp(qg + 1, 0)
        ot = outsb.tile([128, 4 * OD], f32, name="ot", tag="outsb")
        tail_norm(0, ot)
        if qg + 2 < qg_n:
            proj_qk(qt, wq_t, bq_t, qg + 2, pool=pssmall, tag="av")
        tail_norm(1, ot)
        nc.sync.dma_start(
            out[qg * QW:(qg + 1) * QW, :].rearrange(
                "(blk p) d -> p blk d", p=128),
            ot[:].rearrange("p (blk d) -> p blk d", blk=4),
        )


def _get_nc(s=S):
    if s not in _CACHE:
        _CACHE[s] = _build(s)
    return _CACHE[s]


def _shard_inputs(x, Wq, bq, Wk, bk, Wv, bv):
    import ml_dtypes

    bf16 = ml_dtypes.bfloat16
    f32 = np.float32
    ident = np.eye(128, dtype=f32)
    xTb = [np.ascontiguousarray(x[b].T).astype(bf16) for b in range(B)]

    def pack_w(Wm, scale=1.0):
        # [D, OD] transposed weight -> [128, 4*OD] (4 chunks side by side)
        WmT = np.ascontiguousarray(Wm.T * scale).astype(bf16)
        return np.concatenate(
            [WmT[c * 128:(c + 1) * 128, :] for c in range(4)], axis=1)

    wq_s, wk_s, wv_s, misc_s = [], [], [], []
    for hp in range(4):
        r = slice(128 * hp, 128 * hp + 128)
        wq_s.append(pack_w(Wq[r], 0.125))
        wk_s.append(pack_w(Wk[r]))
        wv_s.append(pack_w(Wv[r]))
        m = np.zeros((128, 258), f32)
        m[:, 0:128] = np.tile(bv[r][None, :], (128, 1))
        m[:, 128:256] = ident
        m[:, 256] = bk[r]
        m[:, 257] = bq[r] * 0.125
        misc_s.append(m)
    in_maps = []
    for c in range(8):
        b, hp = divmod(c, 4)
        in_maps.append({
            "xT": xTb[b],
            "wqT": wq_s[hp],
            "wkT": wk_s[hp],
            "wvT": wv_s[hp],
            "misc": misc_s[hp],
        })
    return in_maps


def kernel(x, Wq, bq, Wk, bk, Wv, bv, _trace=False):
    from concourse.bass_utils import run_bass_kernel_spmd

    x = np.asarray(x, dtype=np.float32)
    Wq = np.asarray(Wq, dtype=np.float32)
    bq = np.asarray(bq, dtype=np.float32)
    Wk = np.asarray(Wk, dtype=np.float32)
    bk = np.asarray(bk, dtype=np.float32)
    Wv = np.asarray(Wv, dtype=np.float32)
    bv = np.asarray(bv, dtype=np.float32)

    nc = _get_nc(S)
    in_maps = _shard_inputs(x, Wq, bq, Wk, bk, Wv, bv)
    try:
        res = run_bass_kernel_spmd(nc, in_maps, core_ids=list(range(8)),
                                   trace=_trace)
    except (ModuleNotFoundError, ImportError):
        import os
        os.environ["BASS_NEVER_TRACE"] = "1"
        res = run_bass_kernel_spmd(nc, in_maps, core_ids=list(range(8)),
                                   trace=False)
    kernel._last_results = res

    out = np.empty((B, S, D), dtype=np.float32)
    for c in range(8):
        b, hp = divmod(c, 4)
        out[b, :, 128 * hp:128 * hp + 128] = res.results[c]["out"]
    return out


# revision 24
# speedup vs baseline: 1.5121x; 1.5121x over previous
"""Multi-head attention (B=2, S=4096, D=512, H=8, HD=64, fp32) on 8 TRN2 cores.

Sharding: core c -> batch b = c//4, local head pair hp = c%4 (global heads
2*hp, 2*hp+1).  Attention is head-independent: no cross-core communication.

Design (v4 -- fp8 DoubleRow attn@V + ACT/DVE exp split + streamed
double-buffering):

  QT = wqT.T @ xT -> [128(2 heads x 64), S] bf16 (score scale folded into wqT)
  KT likewise.  V  = x @ Wv.T + bv, written as fp8e4m3 into a DoubleRow-
  interleaved stationary layout v_dr[128k, head, kcpair, t, m] with m in
  0..127: cols 0-63 = head dims, col 64 = 1.0 (denominator), 65-127 = 0.

  per q-group of 512 queries, per unit (kcpair j of 256 keys, head h):
    scoresT chunk: 2 bf16 matmuls -> PSUM [128, 1024] (keys t=0/t=1 halves)
    exp: either ACT activation Exp (PSUM f32 -> SBUF fp8e4m3), or a
      one-instruction DVE Schraudolph: int8 = rint(s*8*log2e + 55.2),
      bitcast to fp8e4m3.  The split fraction balances ACT vs DVE occupancy.
    attn@V: ONE DoubleRow fp8 matmul per unit: lhsT = v_dr slice
      [128, 2, 128], rhs = ex [128, 2, 512], accumulating av[h] [128, 512]
      over the 16 kcpairs.  Row 64 = softmax denominator (ones column).
      DoubleRow runs at 0.5 PE cycles/row = 4x the bf16 M=65 variant.

  DMA: per-transfer HWDGE overhead is ~625ns, so everything is batched:
  packed weight tiles (1 transfer each), xT column-blocks carrying all 4
  chunks per transfer (split across the SP and gpsimd queues), one misc
  tile for biases/ident, one output DMA per q-group.

  Startup: only K block 0 + Q block 0 + V kcpairs 0-1 precede attention;
  the other K blocks / V chunks are software-pipelined into qg 0's loop.

  Tail per qg: copy av[0:65] to SBUF (emitted early for h0), 4 PE
  transposes into one [128, 260] PSUM tile, ONE strided reciprocal
  [128, 4] (DVE), per-block scale on ACT (activation Copy, scale=rs AP),
  one [128, 512] DMA.  The next q-group's first two scores+exps are
  peeled into the tail so both exp engines run through it; Q proj is
  emitted two q-groups ahead.

  qt/kt/v_dr are double-buffered across rep bodies so streamed execution
  overlaps body n+1's projections with body n's attention (rep=1
  single-call path is unchanged).
"""

import numpy as np

B, S, D, H = 2, 4096, 512, 8
HD = D // H          # 64
OD = 128             # output dims per core (2 heads)
QW = 512             # query group width

A_SCH = 11.541560327111707   # 8 * log2(e)
B_SCH = 55.2                 # 7*8 (fp8e4m3 bias) - 0.8 (tuned offset)

PSBIG_BUFS = 3
EXPS_BUFS = 12
OUTSB_BUFS = 4
OUTT_BUFS = 2
USE_GPSIMD_DMA = True

_CACHE = {}


def _build(s=S, rep=1, sden=32, snum=13, qg_limit=None):
    global PSBIG_BUFS
    import concourse.bacc as bacc
    import concourse.mybir as mybir
    import concourse.tile as tile

    f32 = mybir.dt.float32
    bf16 = mybir.dt.bfloat16
    fp8 = mybir.dt.float8e4
    Exp = mybir.ActivationFunctionType.Exp

    nc = bacc.Bacc(None, target_bir_lowering=False)

    xT = nc.dram_tensor("xT", [D, s], bf16, kind="ExternalInput")
    wqT = nc.dram_tensor("wqT", [128, 4 * OD], bf16, kind="ExternalInput")
    wkT = nc.dram_tensor("wkT", [128, 4 * OD], bf16, kind="ExternalInput")
    wvT = nc.dram_tensor("wvT", [128, 4 * OD], bf16, kind="ExternalInput")
    # misc: cols 0:128 bvb, 128:256 ident, 256 bk, 257 bq
    misc = nc.dram_tensor("misc", [128, 258], f32, kind="ExternalInput")
    out = nc.dram_tensor("out", [s, OD], f32, kind="ExternalOutput")

    with tile.TileContext(nc) as tc:
        with (
            tc.tile_pool(name="persist", bufs=1) as persist,
            tc.tile_pool(name="exps", bufs=EXPS_BUFS) as exps,
            tc.tile_pool(name="outsb", bufs=OUTSB_BUFS) as outsb,
            tc.tile_pool(name="outt", bufs=OUTT_BUFS) as outtp,
            tc.tile_pool(name="psbig", bufs=PSBIG_BUFS, space="PSUM") as psbig,
            tc.tile_pool(name="pssmall", bufs=2, space="PSUM") as pssmall,
            tc.tile_pool(name="pstp", bufs=2, space="PSUM") as pstp,
        ):
            # Warm the ScalarE exp table while input DMAs stream.
            warm = persist.tile([1, 1], f32, name="warm", tag="warm")
            nc.vector.memset(warm[:], 0.0)
            nc.scalar.activation(warm[:], warm[:], Exp)

            wq_t = persist.tile([128, 4 * OD], bf16, name="wq_t", tag="wq")
            wk_t = persist.tile([128, 4 * OD], bf16, name="wk_t", tag="wk")
            wv_t = persist.tile([128, 4 * OD], bf16, name="wv_t", tag="wv")
            misc_t = persist.tile([128, 258], f32, name="misc_t", tag="misc")
            # weights ride the Activation HWDGE queue (idle this early)
            # so xT block 0 heads the SP queue and K proj starts sooner.
            nc.scalar.dma_start(wk_t[:], wkT[:])
            nc.scalar.dma_start(misc_t[:], misc[:])
            nc.scalar.dma_start(wq_t[:], wqT[:])

            # xT: one SBUF tile [128, 4*s]; each transfer carries a 512-col
            # block of all 4 chunks, split across the SP / gpsimd queues.
            xt_all = persist.tile([128, 4 * s], bf16, name="xt_all",
                                  tag="xt")
            xt_v = xt_all[:].rearrange("p (c n) -> p c n", c=4)
            xT_v = xT[:].rearrange("(c p) n -> p c n", c=4)
            # block 0 split per-chunk across three queues: lands ~2.5us
            # so K-proj block 0 starts immediately.
            for c, eng in ((0, nc.sync), (1, nc.sync), (2, nc.scalar),
                           (3, nc.gpsimd)):
                eng.dma_start(xt_v[:, c, 0:QW], xT_v[:, c, 0:QW])
            for b in range(1, s // QW):
                eng = nc.gpsimd if (USE_GPSIMD_DMA and b % 2 == 1) else nc.sync
                eng.dma_start(xt_v[:, :, b * QW:(b + 1) * QW],
                              xT_v[:, :, b * QW:(b + 1) * QW])
            nc.sync.dma_start(wv_t[:], wvT[:])

            # qt/kt/v_dr double-buffered across bodies so a streamed
            # (rep-chained) execution overlaps body n+1's projections with
            # body n's attention.  v_dr: DoubleRow-interleaved V (+ones
            # +zeros): per head h, kcpair j, t in (0,1): cols
            # h*s + j*256 + t*128 + (0..127).
            qts = [persist.tile([128, s], bf16, name=f"qt{i}", tag=f"qt{i}")
                   for i in range(min(rep, 2))]
            kts = [persist.tile([128, s], bf16, name=f"kt{i}", tag=f"kt{i}")
                   for i in range(min(rep, 2))]
            v_drs = [persist.tile([128, 2 * s], fp8, name=f"v_dr{i}",
                                  tag=f"v_dr{i}") for i in range(min(rep, 2))]

            for i in range(rep):
                _emit_body(nc, tc, mybir, s, qts[i % 2 if rep > 1 else 0],
                           kts[i % 2 if rep > 1 else 0],
                           v_drs[i % 2 if rep > 1 else 0], xt_all,
                           wq_t, wk_t, wv_t, misc_t, out,
                           persist, exps, outsb, outtp, psbig, pssmall,
                           pstp, sden, snum, qg_limit)

    nc.compile()
    return nc


def _emit_body(nc, tc, mybir, s, qt, kt, v_dr, xt_all, wq_t, wk_t, wv_t,
               misc_t, out, persist, exps, outsb, outtp, psbig, pssmall,
               pstp, sden, snum, qg_limit=None):
    f32 = mybir.dt.float32
    fp8 = mybir.dt.float8e4
    i8 = mybir.dt.int8
    Exp = mybir.ActivationFunctionType.Exp
    Mult = mybir.AluOpType.mult
    Add = mybir.AluOpType.add
    DR = mybir.MatmulPerfMode.DoubleRow

    qg_n = qg_limit if qg_limit else (s // QW)
    kc_n = s // 128         # 32 key chunks of 128
    jp_n = kc_n // 2        # 16 kcpairs of 256
    sb_n = s // QW

    bvb = misc_t[:, 0:128]
    id_t = misc_t[:, 128:256]
    bk_t = misc_t[:, 256:257]
    bq_t = misc_t[:, 257:258]

    def xts(c, lo, hi):
        return xt_all[:, c * s + lo:c * s + hi]

    def proj_qk(dst, w_t, b_t, sb, pool=None, tag="sc"):
        ps = (pool or psbig).tile([128, QW], f32, name="ps_proj", tag=tag)
        for c in range(4):
            nc.tensor.matmul(
                ps[:],
                lhsT=w_t[:, c * OD:(c + 1) * OD],
                rhs=xts(c, sb * QW, (sb + 1) * QW),
                start=(c == 0),
                stop=(c == 3),
            )
        nc.vector.tensor_scalar_add(
            dst[:, sb * QW:(sb + 1) * QW], ps[:], b_t
        )

    def proj_v(sb):
        # V projection chunk -> DoubleRow layout (via psbig so it never
        # contends with the av accumulators during attention).
        ps = psbig.tile([128, 128], f32, name="ps_vp", tag="sc")
        for c in range(4):
            nc.tensor.matmul(
                ps[:],
                lhsT=xts(c, sb * 128, (sb + 1) * 128),
                rhs=wv_t[:, c * OD:(c + 1) * OD],
                start=(c == 0),
                stop=(c == 3),
            )
        # one strided add writes both heads' 64 dims (fp8 out)
        col = (sb // 2) * 256 + (sb % 2) * 128
        nc.vector.tensor_tensor(
            v_dr[:].rearrange("p (h c) -> p h c", h=2)[:, :, col:col + 64],
            ps[:].rearrange("p (h c) -> p h c", h=2),
            bvb.rearrange("p (h c) -> p h c", h=2),
            Add,
        )

    # Minimal prelude: K block 0, Q block 0, V chunks 0-3 (kcpairs 0-1).
    # Remaining K blocks / V chunks (and Q block 1) are software-pipelined
    # into qg 0's unit loop just ahead of first use, so ScalarE starts
    # exp'ing early.
    nc.gpsimd.memset(v_dr[:], 0.0)
    nc.gpsimd.memset(
        v_dr[:].rearrange("p (n m) -> p n m", m=128)[:, :, 64:65], 1.0
    )
    proj_qk(kt, wk_t, bk_t, 0)
    proj_qk(qt, wq_t, bq_t, 0)
    for sb in (0, 1, 2, 3):
        proj_v(sb)

    def emit_av(av, ex, u):
        j, h = u >> 1, u & 1
        nc.tensor.matmul(
            av[h][:],
            lhsT=v_dr[:].rearrange(
                "p (h2 j2 t m) -> p h2 j2 t m", h2=2, j2=jp_n, t=2
            )[:, h, j],
            rhs=ex[:].rearrange("p (t n) -> p t n", t=2),
            start=(j == 0),
            stop=(j == jp_n - 1),
            perf_mode=DR,
        )

    def emit_scores_exp(qg, u):
        j, h = u >> 1, u & 1
        ps = psbig.tile([128, 2 * QW], f32, name="ps_sc", tag="sc")
        for t in (0, 1):
            kc = 2 * j + t
            nc.tensor.matmul(
                ps[:, t * QW:(t + 1) * QW],
                lhsT=kt[h * HD:(h + 1) * HD,
                        kc * 128:(kc + 1) * 128],
                rhs=qt[h * HD:(h + 1) * HD,
                       qg * QW:(qg + 1) * QW],
                start=True,
                stop=True,
            )
        ex = exps.tile([128, 2 * QW], fp8, name="ex", tag="exp")
        if qg == 0:
            # early qg0 DVE is busy with projection copies; add DVE-exp
            # units only in the second half once those drain
            dve_exp = (u % 16) == 7 or (u >= 24 and u % 2 == 1)
        else:
            dve_exp = (u * snum) % sden < snum
        if dve_exp:
            nc.vector.tensor_scalar(
                ex[:].bitcast(i8), ps[:], A_SCH, B_SCH, Mult, Add
            )
        else:
            nc.scalar.activation(ex[:], ps[:], Exp)
        return ex

    # ---- attention ----
    # Software-pipelined: each unit's attn@V matmul is emitted one unit
    # late so the PE computes the next scores while the exp runs; unit 0's
    # scores+exp are peeled into the previous q-group's tail.
    peeled = []
    for qg in range(qg_n):
        av = [pssmall.tile([128, QW], f32, name="av", tag="av")
              for _ in (0, 1)]
        outt_h = [None, None]

        def tail_copy(h):
            outt = outtp.tile([65, QW], f32, name="outt", tag="outt")
            nc.vector.tensor_copy(outt[:], av[h][0:65, :])
            outt_h[h] = outt

        def tail_norm(h, ot):
            outt = outt_h[h]
            tp = pssmall.tile([128, 260], f32, name="tp", tag="av")
            tpv = tp[:].rearrange("p (blk c) -> p blk c", blk=4)
            for blk in range(4):
                nc.tensor.transpose(
                    tpv[:, blk],
                    outt[:, blk * 128:(blk + 1) * 128],
                    id_t[0:65, 0:65],
                )
            rs = outsb.tile([128, 4], f32, name="rs", tag="rs")
            nc.vector.reciprocal(rs[:], tpv[:, :, 64:65])
            for blk in range(4):
                nc.scalar.activation(
                    ot[:, blk * 128 + h * HD:blk * 128 + (h + 1) * HD],
                    tpv[:, blk, 0:64],
                    mybir.ActivationFunctionType.Copy,
                    bias=0.0, scale=rs[:, blk:blk + 1],
                )

        pend = None
        for u in range(2 * jp_n):
            if qg == 0:
                if u % 4 == 0 and 1 <= u // 4 + 1 <= sb_n - 1:
                    proj_qk(kt, wk_t, bk_t, u // 4 + 1)
                if u == 2:
                    proj_qk(qt, wq_t, bq_t, 1)
                if u % 2 == 0 and u + 4 < kc_n:
                    proj_v(u + 4)
                    proj_v(u + 5)
            if u < len(peeled):
                ex = peeled[u]
            else:
                ex = emit_scores_exp(qg, u)
            if u == 2 * jp_n - 1:
                peeled = []
            if pend is not None:
                emit_av(av, *pend)
                if pend[1] == 2 * jp_n - 2:
                    # h0 accumulation done: its PSUM->SBUF copy overlaps
                    # the last h1 unit.
                    tail_copy(0)
            pend = (ex, u)
        emit_av(av, *pend)
        tail_copy(1)
        # Peel next q-group's first two scores+exps so both exp engines
        # keep running through the tail (qt was projected two tails ago).
        if qg + 1 < qg_n:
            peeled = [emit_scores_exp(qg + 1, 0),
                      emit_scores_exp(qg + 1, 1)]
        ot = outsb.tile([128, 4 * OD], f32, name="ot", tag="outsb")
        tail_norm(0, ot)
        if qg + 2 < qg_n:
            proj_qk(qt, wq_t, bq_t, qg + 2, pool=pssmall, tag="av")
        tail_norm(1, ot)
        nc.sync.dma_start(
            out[qg * QW:(qg + 1) * QW, :].rearrange(
                "(blk p) d -> p blk d", p=128),
            ot[:].rearrange("p (blk d) -> p blk d", blk=4),
        )


def _get_nc(s=S):
    if s not in _CACHE:
        _CACHE[s] = _build(s)
    return _CACHE[s]


def _shard_inputs(x, Wq, bq, Wk, bk, Wv, bv):
    import ml_dtypes

    bf16 = ml_dtypes.bfloat16
    f32 = np.float32
    ident = np.eye(128, dtype=f32)
    xTb = [np.ascontiguousarray(x[b].T).astype(bf16) for b in range(B)]

    def pack_w(Wm, scale=1.0):
        # [D, OD] transposed weight -> [128, 4*OD] (4 chunks side by side)
        WmT = np.ascontiguousarray(Wm.T * scale).astype(bf16)
        return np.concatenate(
            [WmT[c * 128:(c + 1) * 128, :] for c in range(4)], axis=1)

    wq_s, wk_s, wv_s, misc_s = [], [], [], []
    for hp in range(4):
        r = slice(128 * hp, 128 * hp + 128)
        wq_s.append(pack_w(Wq[r], 0.125))
        wk_s.append(pack_w(Wk[r]))
        wv_s.append(pack_w(Wv[r]))
        m = np.zeros((128, 258), f32)
        m[:, 0:128] = np.tile(bv[r][None, :], (128, 1))
        m[:, 128:256] = ident
        m[:, 256] = bk[r]
        m[:, 257] = bq[r] * 0.125
        misc_s.append(m)
    in_maps = []
    for c in range(8):
        b, hp = divmod(c, 4)
        in_maps.append({
            "xT": xTb[b],
            "wqT": wq_s[hp],
            "wkT": wk_s[hp],
            "wvT": wv_s[hp],
            "misc": misc_s[hp],
        })
    return in_maps


def kernel(x, Wq, bq, Wk, bk, Wv, bv, _trace=False):
    from concourse.bass_utils import run_bass_kernel_spmd

    x = np.asarray(x, dtype=np.float32)
    Wq = np.asarray(Wq, dtype=np.float32)
    bq = np.asarray(bq, dtype=np.float32)
    Wk = np.asarray(Wk, dtype=np.float32)
    bk = np.asarray(bk, dtype=np.float32)
    Wv = np.asarray(Wv, dtype=np.float32)
    bv = np.asarray(bv, dtype=np.float32)

    nc = _get_nc(S)
    in_maps = _shard_inputs(x, Wq, bq, Wk, bk, Wv, bv)
    try:
        res = run_bass_kernel_spmd(nc, in_maps, core_ids=list(range(8)),
                                   trace=_trace)
    except (ModuleNotFoundError, ImportError):
        import os
        os.environ["BASS_NEVER_TRACE"] = "1"
        res = run_bass_kernel_spmd(nc, in_maps, core_ids=list(range(8)),
                                   trace=False)
    kernel._last_results = res

    out = np.empty((B, S, D), dtype=np.float32)
    for c in range(8):
        b, hp = divmod(c, 4)
        out[b, :, 128 * hp:128 * hp + 128] = res.results[c]["out"]
    return out
